# revision 61
# baseline (speedup 1.0000x reference)
"""Disentangled multi-head attention (DeBERTa-style) Trainium2 Bass kernel.

Full inputs in, full outputs out. Sharding: batch (B=8) across 8 cores, data
parallel; each core computes all H=8 heads for its batch element.

Math (per batch b):
  q,k,v = x@W? + b?                                   [S, D]
  rel_emb[i,j] = rel_tab[j-i+511]  (Toeplitz: only 1023 distinct rows)
  P_k = rel_tab@Wpk + bpk ; P_q = rel_tab@Wpq + bpq   [1023, D]
  c2c[i,j] = q_i . k_j
  c2p[i,j] = q_i . P_k[j-i+511]  = qP[i, j-i+511],    qP  = q @ P_k^T
  p2c[i,j] = k_j . P_q[j-i+511]  = kPf[j, i-j+511],   kPf = k @ P_qflip^T
  out = softmax((c2c+c2p+p2c)/sqrt(3*64)) @ v ; y = out@Wo + bo

Kernel works in transposed-logits layout logitsT[j, i]:
  c2cT  : matmul(lhsT=khT_chunk, rhs=qhT)
  c2pT  : diag-DMA qP rows (per-partition shifted slice) then PE-transpose
  p2cT  : diag-DMA kPf rows directly (already [j, i])
  softmax: exp on ACT; denominator via ones-column in the AV matmul
  (row 64 of av psum = sum_j expT[j,i]); normalize after AV.

Heads are software-pipelined (head h+1's qP/kPf matmul+evict+diag chains
are emitted before head h's logits/AV phase). Adjacent heads sit on PE row
groups 0-63 / 64-127, so their K=64 matmuls can pack into disjoint row
groups at runtime. Dense matmuls stream float32r (1 cyc/row vs 4 for fp32);
qP/kPf compute only the 640-wide window the diagonal actually reads; the
p2cT diag path runs in bf16.
"""

import math
import os
import sys
import threading

import numpy as np

for _p in ("/opt/trn_rl_repo",):
    if _p not in sys.path and os.path.isdir(_p):
        sys.path.insert(0, _p)

import concourse.bacc as bacc
import concourse.bass as bass
import concourse.mybir as mybir
import concourse.tile as tile
from concourse.ap import AP
from concourse.bass_utils import run_bass_kernel_spmd
from concourse.masks import make_identity

S = 512
D = 512
H = 8
DH = 64
L = 512
W = 2 * L - 1  # 1023
WP = 1024  # padded so fp32r matmuls keep even 512-wide moving dims
WIN = 640  # 639-wide diag window, rounded up
NCORES = 8
SCALE = 1.0 / math.sqrt(3.0 * DH)

F32 = mybir.dt.float32
F32R = mybir.dt.float32r
BF16 = mybir.dt.bfloat16
MM_DT = F32R


def _diag_ap(t, col0, nrows, ncols):
    """Per-partition shifted read: out[p, j] = t[p, col0 - p + j]."""
    rs = t.ap[0][0]
    return AP(t.tensor, t.offset + col0, [[rs - 1, nrows], [1, ncols]])


def _rev_ap(t, ncols):
    """Free-dim reversed view of a [P, ncols] tile/psum AP."""
    rs = t.ap[0][0]
    return AP(t.tensor, t.offset + ncols - 1, [[rs, t.shape[0]], [-1, ncols]])


def build_program():
    nc = bacc.Bacc(trn_type="TRN2")

    x = nc.dram_tensor("x", [S, D], MM_DT, kind="ExternalInput")
    Wq = nc.dram_tensor("Wq", [D, D], MM_DT, kind="ExternalInput")
    bq = nc.dram_tensor("bq", [D], F32, kind="ExternalInput")
    Wk = nc.dram_tensor("Wk", [D, D], MM_DT, kind="ExternalInput")
    bk = nc.dram_tensor("bk", [D], F32, kind="ExternalInput")
    Wv = nc.dram_tensor("Wv", [D, D], MM_DT, kind="ExternalInput")
    bv = nc.dram_tensor("bv", [D], F32, kind="ExternalInput")
    rel_tab = nc.dram_tensor("rel_tab", [W, D], MM_DT, kind="ExternalInput")
    Wpk = nc.dram_tensor("Wpk", [D, D], MM_DT, kind="ExternalInput")
    bpk = nc.dram_tensor("bpk", [D], F32, kind="ExternalInput")
    Wpq = nc.dram_tensor("Wpq", [D, D], MM_DT, kind="ExternalInput")
    bpq = nc.dram_tensor("bpq", [D], F32, kind="ExternalInput")
    Wo = nc.dram_tensor("Wo", [D, D], MM_DT, kind="ExternalInput")
    bo = nc.dram_tensor("bo", [D], F32, kind="ExternalInput")
    y = nc.dram_tensor("y", [S, D], F32, kind="ExternalOutput")

    with tile.TileContext(nc) as tc:
        with (
            tc.tile_pool(name="const", bufs=1) as constp,
            tc.tile_pool(name="persist", bufs=1) as persist,
        ):
            ident = constp.tile([128, 128], F32, name="ident")
            make_identity(nc, ident)
            ident_r = constp.tile([128, 128], MM_DT, name="ident_r")
            nc.scalar.copy(ident_r[:], ident[:])

            # =========================== phase A ===========================
            with (
                tc.tile_pool(name="wload", bufs=1) as wload,
                tc.tile_pool(name="ps_xt", bufs=1, space="PSUM") as ps_xt,
                tc.tile_pool(name="ps_rt", bufs=2, space="PSUM") as ps_rt,
                tc.tile_pool(name="ps_pj", bufs=3, space="PSUM") as ps_pj,
            ):

                def load_packed(dram, nrows, name, dt=F32, eng=None):
                    """One big DMA: [nrows, D] row chunks packed side by side
                    in the free dim; chunk c = tile[:, c*D:(c+1)*D]."""
                    eng = eng or nc.sync
                    nch = (nrows + 127) // 128
                    t = wload.tile([128, nch * D], dt, name=name)
                    full = nrows // 128
                    flat = dram[:, :].rearrange("a b -> (a b)")
                    rs = t.ap[0][0]
                    if full:
                        eng.dma_start(
                            AP(t.tensor, t.offset, [[rs, 128], [D, full], [1, D]]),
                            AP(flat.tensor, 0, [[D, 128], [128 * D, full], [1, D]]),
                        )
                    if full < nch:  # remainder rows
                        p = nrows - full * 128
                        eng.dma_start(
                            t[:p, full * D : full * D + D],
                            dram[full * 128 : nrows, :],
                        )
                    return [t[:, c * D : (c + 1) * D] for c in range(nch)]

                def load_chunked(dram, nrows, name, eng, dt=MM_DT):
                    nch = (nrows + 127) // 128
                    t = wload.tile([128, nch * D], dt, name=name)
                    for c in range(nch):
                        p = min(128, nrows - c * 128)
                        eng.dma_start(
                            t[:p, c * D : c * D + D],
                            dram[c * 128 : c * 128 + p, :],
                        )
                    return [t[:, c * D : (c + 1) * D] for c in range(nch)]


                bv_bc = constp.tile([128, D], F32, name="bv_bc")
                nc.sync.dma_start(bv_bc[:], AP(bv[:].tensor, 0, [[0, 128], [1, D]]))

                x_t = load_chunked(x, S, "x", eng=nc.scalar)
                rel_t = load_chunked(rel_tab, W, "rel", eng=nc.scalar)
                Wq_t = load_chunked(Wq, D, "Wq", eng=nc.sync)
                Wk_t = load_chunked(Wk, D, "Wk", eng=nc.sync)
                Wv_t = load_chunked(Wv, D, "Wv", eng=nc.sync)
                Wpk_t = load_chunked(Wpk, D, "Wpk", eng=nc.sync)
                Wpq_t = load_chunked(Wpq, D, "Wpq", eng=nc.sync)

                def load_bias_cols(dram, name):
                    t = constp.tile([128, 4], F32, name=name)
                    rs = t.ap[0][0]
                    nc.sync.dma_start(
                        AP(t.tensor, t.offset, [[rs, 128], [1, 4], [1, 1]]),
                        AP(dram[:].tensor, 0, [[1, 128], [128, 4], [1, 1]]),
                    )
                    return [t[:, c : c + 1] for c in range(4)]

                bq_t = load_bias_cols(bq, "bq")
                bk_t = load_bias_cols(bk, "bk")
                bpk_t = load_bias_cols(bpk, "bpk")
                bpq_t = load_bias_cols(bpq, "bpq")

                # ---- xT via PE transpose ----
                xT_t = []
                for ec in range(4):
                    ps = ps_xt.tile([128, S], F32, name="ps_xtt", tag="ps_xtt")
                    for sc in range(4):
                        nc.tensor.matmul(
                            ps[:, sc * 128 : (sc + 1) * 128].bitcast(MM_DT),
                            x_t[sc][:, ec * 128 : (ec + 1) * 128],
                            ident_r[:],
                            is_transpose=True,
                            start=(sc == 0),
                            stop=(sc == 3),
                        )
                    t = wload.tile([128, S], MM_DT, name=f"xT{ec}")
                    nc.scalar.copy(t[:], ps[:])
                    xT_t.append(t)

                # ---- qT, kT (per-partition bias), v (broadcast bias) ----
                def proj_T(W_t, b_t, name):
                    out = []
                    for dcc in range(4):
                        ps = ps_pj.tile([128, S], F32, name="ps_prj", tag="ps_prj")
                        for ec in range(4):
                            nc.tensor.matmul(
                                ps[:],
                                W_t[ec][:, dcc * 128 : (dcc + 1) * 128],
                                xT_t[ec][:],
                                start=(ec == 0),
                                stop=(ec == 3),
                            )
                        t = persist.tile([128, S], MM_DT, name=f"{name}{dcc}")
                        nc.scalar.activation(
                            t[:],
                            ps[:],
                            mybir.ActivationFunctionType.Identity,
                            bias=b_t[dcc],
                        )
                        out.append(t)
                    return out

                qT_t = proj_T(Wq_t, bq_t, "qT")
                kT_t = proj_T(Wk_t, bk_t, "kT")

                v_t = []
                for sc in range(4):
                    ps = ps_pj.tile([128, D], F32, name="ps_vv", tag="ps_prj")
                    for ec in range(4):
                        nc.tensor.matmul(
                            ps[:],
                            xT_t[ec][:, sc * 128 : (sc + 1) * 128],
                            Wv_t[ec][:],
                            start=(ec == 0),
                            stop=(ec == 3),
                        )
                    t = persist.tile([128, D], F32, name=f"v{sc}")
                    nc.vector.tensor_add(t[:], ps[:], bv_bc[:])
                    v_t.append(t)

                # all heads' ones-augmented v built once, off the head loop
                vh_all = []
                for h in range(H):
                    row = []
                    for sc in range(4):
                        va = persist.tile(
                            [128, DH + 1], MM_DT, name=f"vaug{h}_{sc}"
                        )
                        nc.vector.tensor_copy(
                            va[:, 0:DH], v_t[sc][:, h * DH : h * DH + DH]
                        )
                        nc.vector.memset(va[:, DH : DH + 1].bitcast(F32), 1.0)
                        row.append(va)
                    vh_all.append(row)

                # ---- rel_tabT via PE transpose: [512, 1023] ----
                relT_t = []
                for dc in range(4):
                    ps = ps_rt.tile([128, WP], F32, name="ps_rtt", tag="ps_rtt")
                    for rc in range(8):
                        # last chunk has 127 valid rows; transpose all 128 --
                        # the garbage column lands in the pad col 1023, which
                        # the eviction below never reads.
                        nc.tensor.matmul(
                            ps[:, rc * 128 : rc * 128 + 128].bitcast(MM_DT),
                            rel_t[rc][:, dc * 128 : (dc + 1) * 128],
                            ident_r[:],
                            is_transpose=True,
                            start=(rc % 4 == 0),
                            stop=(rc % 4 == 3),
                        )
                    t = wload.tile([128, WP], MM_DT, name=f"relT{dc}")
                    if dc % 2 == 0:
                        nc.vector.tensor_copy(t[:, 0:W], ps[:, 0:W])
                    else:
                        nc.scalar.copy(t[:, 0:W], ps[:, 0:W])
                    nc.vector.memset(t[:, W:WP].bitcast(F32), 0.0)
                    relT_t.append(t)

                # ---- P_kT [512, 1024] and P_qT flipped ----
                def posproj_chunk(W_t, b_t, name, flip, dcc):
                    ps = ps_rt.tile([128, WP], F32, name="ps_pp", tag="ps_rtt")
                    for n0 in (0, 512):
                        for ec in range(4):
                            nc.tensor.matmul(
                                ps[:, n0 : n0 + 512],
                                W_t[ec][:, dcc * 128 : (dcc + 1) * 128],
                                relT_t[ec][:, n0 : n0 + 512],
                                start=(ec == 0),
                                stop=(ec == 3),
                            )
                    t = persist.tile([128, WP], MM_DT, name=f"{name}{dcc}")
                    if flip:
                        nc.scalar.activation(
                            t[:, 0:W],
                            _rev_ap(ps, W),
                            mybir.ActivationFunctionType.Identity,
                            bias=b_t[dcc],
                        )
                        nc.vector.memset(t[:, W:WP].bitcast(F32), 0.0)
                    else:
                        nc.scalar.activation(
                            t[:],
                            ps[:],
                            mybir.ActivationFunctionType.Identity,
                            bias=b_t[dcc],
                        )
                    return t

                PkT_t, PqTf_t = [], []
                for dcc in range(4):
                    PkT_t.append(
                        posproj_chunk(Wpk_t, bpk_t, "PkT", False, dcc)
                    )
                    PqTf_t.append(
                        posproj_chunk(Wpq_t, bpq_t, "PqTf", True, dcc)
                    )

                bo_bc = constp.tile([128, D], F32, name="bo_bc")
                nc.sync.dma_start(bo_bc[:], AP(bo[:].tensor, 0, [[0, 128], [1, D]]))
                Wo_pk = persist.tile([DH, H * D], MM_DT, name="Wo_pk")
                for h in range(H):
                    nc.sync.dma_start(
                        Wo_pk[:, h * D : (h + 1) * D],
                        Wo[h * DH : (h + 1) * DH, :],
                    )
                Wo_h = [Wo_pk[:, h * D : (h + 1) * D] for h in range(H)]

            # =========================== phase B ===========================
            # Heads in pairs: even head on partitions 0-63, odd on 64-127,
            # so their K=64 matmuls pack into disjoint PE row groups.
            with (
                tc.tile_pool(name="hwork", bufs=2) as hwork,
                tc.tile_pool(name="ps_h", bufs=6, space="PSUM") as ps_h,
                tc.tile_pool(name="ps_av", bufs=2, space="PSUM") as ps_av,
            ):

                def qp_pipeline(thT, PhT, tag, dt):
                    """qP/kPf window halves -> evict -> diag read.
                    Only the 640-wide window [384-i0, 1024-i0) of qP row-chunk
                    ic is ever read by the diagonal, so compute just that:
                    one N=512 and one N=128 matmul per chunk.
                    Returns 4 diag tiles [128, S]."""
                    outs = []
                    for ic in range(4):
                        i0 = ic * 128
                        pa = ps_h.tile([128, 512], F32, name="ps_qpa", tag="ps_h")
                        nc.tensor.matmul(
                            pa[:],
                            thT[:, i0 : i0 + 128],
                            PhT[:, 384 - i0 : 896 - i0],
                        )
                        pb = ps_h.tile([128, 128], F32, name="ps_qpb", tag="ps_h")
                        nc.tensor.matmul(
                            pb[:],
                            thT[:, i0 : i0 + 128],
                            PhT[:, 896 - i0 : 1024 - i0],
                        )
                        sb = hwork.tile([128, WIN], dt, name=f"{tag}sb{ic}", bufs=1)
                        if ic % 2 == 0:
                            nc.vector.tensor_copy(sb[:, 0:512], pa[:])
                            nc.scalar.copy(sb[:, 512:640], pb[:])
                        else:
                            nc.scalar.copy(sb[:, 0:512], pa[:])
                            nc.vector.tensor_copy(sb[:, 512:640], pb[:])
                        dg = hwork.tile([128, S], dt, name=f"{tag}dg{ic}", bufs=3)
                        nc.sync.dma_start(dg[:], _diag_ap(sb, 127, 128, S))
                        outs.append(dg)
                    return outs

                outT_h = [None] * H

                def head_views(h):
                    dc, hs = h // 2, (h % 2) * DH
                    return (
                        qT_t[dc][hs : hs + DH, :],
                        kT_t[dc][hs : hs + DH, :],
                        PkT_t[dc][hs : hs + DH, :],
                        PqTf_t[dc][hs : hs + DH, :],
                    )

                def emit_pipes(h):
                    qhT, khT, PkhT, PqhTf = head_views(h)
                    c2p = qp_pipeline(qhT, PkhT, "qp", MM_DT)
                    p2cT = qp_pipeline(khT, PqhTf, "kp", BF16)
                    return c2p, p2cT

                pipes = {0: emit_pipes(0)}
                for h in range(H):
                    qhT, khT, PkhT, PqhTf = head_views(h)
                    if h + 1 < H:
                        pipes[h + 1] = emit_pipes(h + 1)
                    c2p, p2cT = pipes.pop(h)

                    ex = []
                    for jc in range(4):
                        ps = ps_h.tile([128, S], F32, name="ps_lg", tag="ps_h")
                        nc.tensor.matmul(
                            ps[:],
                            khT[:, jc * 128 : (jc + 1) * 128],
                            qhT[:],
                            start=True,
                            stop=False,
                        )
                        for ic in range(4):
                            nc.tensor.matmul(
                                ps[:, ic * 128 : (ic + 1) * 128].bitcast(MM_DT),
                                c2p[ic][:, jc * 128 : (jc + 1) * 128],
                                ident_r[:],
                                is_transpose=True,
                                start=False,
                                stop=(ic == 3),
                            )
                        et = hwork.tile([128, S], MM_DT, name=f"ex{jc}", bufs=3)
                        nc.vector.tensor_add(et[:], ps[:], p2cT[jc][:])
                        nc.scalar.activation(
                            et[:],
                            et[:],
                            mybir.ActivationFunctionType.Exp,
                            scale=SCALE,
                        )
                        ex.append(et[:])

                    av = ps_av.tile([DH + 1, S], F32, name="ps_avt", tag="ps_avt")
                    for jc in range(4):
                        nc.tensor.matmul(
                            av[:],
                            vh_all[h][jc][:],
                            ex[jc],
                            start=(jc == 0),
                            stop=(jc == 3),
                        )
                    zr = hwork.tile([DH + 1, S], F32, name="zrow")
                    nc.vector.reciprocal(zr[DH : DH + 1, :], av[DH : DH + 1, :])
                    rbc = hwork.tile([DH, S], F32, name="rbc")
                    rs_zr = zr.ap[0][0]
                    nc.sync.dma_start(
                        rbc[:],
                        AP(zr.tensor, zr.offset + DH * rs_zr,
                           [[rs_zr, 1], [0, DH], [1, S]]),
                    )
                    ot = persist.tile([DH, S], MM_DT, name=f"outT{h}")
                    nc.vector.tensor_mul(ot[:], av[0:DH, :], rbc[:])
                    outT_h[h] = ot

                # ======================= phase C ==========================
                ysb = hwork.tile([128, 4 * D], F32, name="ysb", bufs=1)
                for sc in range(4):
                    ps = ps_h.tile([128, D], F32, name="ps_y", tag="ps_h")
                    for h in range(H):
                        nc.tensor.matmul(
                            ps[:],
                            outT_h[h][:, sc * 128 : (sc + 1) * 128],
                            Wo_h[h],
                            start=(h == 0),
                            stop=(h == 7),
                        )
                    nc.vector.tensor_add(
                        ysb[:, sc * D : (sc + 1) * D], ps[:], bo_bc[:]
                    )
                    nc.sync.dma_start(
                        y[sc * 128 : (sc + 1) * 128, :],
                        ysb[:, sc * D : (sc + 1) * D],
                    )

    nc.compile()
    return nc


_cache_lock = threading.Lock()
_cached_nc = None


def _get_program():
    global _cached_nc
    with _cache_lock:
        if _cached_nc is None:
            _cached_nc = build_program()
    return _cached_nc


def kernel(**inputs):
    x = np.ascontiguousarray(np.asarray(inputs["x"], dtype=np.float32))
    B = x.shape[0]
    assert x.shape == (B, S, D)

    weights = {
        k: np.ascontiguousarray(np.asarray(inputs[k], dtype=np.float32))
        for k in (
            "Wq", "bq", "Wk", "bk", "Wv", "bv",
            "rel_tab", "Wpk", "bpk", "Wpq", "bpq", "Wo", "bo",
        )
    }

    nc = _get_program()
    in_maps = [{"x": x[c], **weights} for c in range(NCORES)]
    res = run_bass_kernel_spmd(nc, in_maps, core_ids=list(range(NCORES)))
    out = np.stack([res.results[c]["y"] for c in range(NCORES)], axis=0)
    return out.astype(np.float32)


if __name__ == "__main__":
    rng = np.random.default_rng(0)
    ins = {
        "x": rng.standard_normal((NCORES, S, D), dtype=np.float32),
        "rel_tab": rng.standard_normal((W, D), dtype=np.float32),
    }
    for nm in ("Wq", "Wk", "Wv", "Wpk", "Wpq", "Wo"):
        ins[nm] = rng.standard_normal((D, D), dtype=np.float32) * 0.04
    for nm in ("bq", "bk", "bv", "bpk", "bpq", "bo"):
        ins[nm] = rng.standard_normal(D).astype(np.float32) * 0.01
    out = kernel(**ins)
    print("ran:", out.shape, out.dtype, np.abs(out).max())



# revision 63
# speedup vs baseline: 1.0013x; 1.0013x over previous
"""Disentangled multi-head attention (DeBERTa-style) Trainium2 Bass kernel.

Full inputs in, full outputs out. Sharding: batch (B=8) across 8 cores, data
parallel; each core computes all H=8 heads for its batch element.

Math (per batch b):
  q,k,v = x@W? + b?                                   [S, D]
  rel_emb[i,j] = rel_tab[j-i+511]  (Toeplitz: only 1023 distinct rows)
  P_k = rel_tab@Wpk + bpk ; P_q = rel_tab@Wpq + bpq   [1023, D]
  c2c[i,j] = q_i . k_j
  c2p[i,j] = q_i . P_k[j-i+511]  = qP[i, j-i+511],    qP  = q @ P_k^T
  p2c[i,j] = k_j . P_q[j-i+511]  = kPf[j, i-j+511],   kPf = k @ P_qflip^T
  out = softmax((c2c+c2p+p2c)/sqrt(3*64)) @ v ; y = out@Wo + bo

Kernel works in transposed-logits layout logitsT[j, i]:
  c2cT  : matmul(lhsT=khT_chunk, rhs=qhT)
  c2pT  : diag-DMA qP rows (per-partition shifted slice) then PE-transpose
  p2cT  : diag-DMA kPf rows directly (already [j, i])
  softmax: exp on ACT; denominator via ones-column in the AV matmul
  (row 64 of av psum = sum_j expT[j,i]); normalize after AV.

Heads are software-pipelined (head h+1's qP/kPf matmul+evict+diag chains
are emitted before head h's logits/AV phase). Adjacent heads sit on PE row
groups 0-63 / 64-127, so their K=64 matmuls can pack into disjoint row
groups at runtime. Dense matmuls stream float32r (1 cyc/row vs 4 for fp32);
qP/kPf compute only the 640-wide window the diagonal actually reads; the
p2cT diag path runs in bf16.
"""

import math
import os
import sys
import threading

import numpy as np

for _p in ("/opt/trn_rl_repo",):
    if _p not in sys.path and os.path.isdir(_p):
        sys.path.insert(0, _p)

import concourse.bacc as bacc
import concourse.bass as bass
import concourse.mybir as mybir
import concourse.tile as tile
from concourse.ap import AP
from concourse.bass_utils import run_bass_kernel_spmd
from concourse.masks import make_identity

S = 512
D = 512
H = 8
DH = 64
L = 512
W = 2 * L - 1  # 1023
WP = 1024  # padded so fp32r matmuls keep even 512-wide moving dims
WIN = 640  # 639-wide diag window, rounded up
NCORES = 8
SCALE = 1.0 / math.sqrt(3.0 * DH)

F32 = mybir.dt.float32
F32R = mybir.dt.float32r
BF16 = mybir.dt.bfloat16
MM_DT = F32R


def _diag_ap(t, col0, nrows, ncols):
    """Per-partition shifted read: out[p, j] = t[p, col0 - p + j]."""
    rs = t.ap[0][0]
    return AP(t.tensor, t.offset + col0, [[rs - 1, nrows], [1, ncols]])


def _rev_ap(t, ncols):
    """Free-dim reversed view of a [P, ncols] tile/psum AP."""
    rs = t.ap[0][0]
    return AP(t.tensor, t.offset + ncols - 1, [[rs, t.shape[0]], [-1, ncols]])


def build_program():
    nc = bacc.Bacc(trn_type="TRN2")

    x = nc.dram_tensor("x", [S, D], MM_DT, kind="ExternalInput")
    Wq = nc.dram_tensor("Wq", [D, D], MM_DT, kind="ExternalInput")
    bq = nc.dram_tensor("bq", [D], F32, kind="ExternalInput")
    Wk = nc.dram_tensor("Wk", [D, D], MM_DT, kind="ExternalInput")
    bk = nc.dram_tensor("bk", [D], F32, kind="ExternalInput")
    Wv = nc.dram_tensor("Wv", [D, D], MM_DT, kind="ExternalInput")
    bv = nc.dram_tensor("bv", [D], F32, kind="ExternalInput")
    rel_tab = nc.dram_tensor("rel_tab", [W, D], MM_DT, kind="ExternalInput")
    Wpk = nc.dram_tensor("Wpk", [D, D], MM_DT, kind="ExternalInput")
    bpk = nc.dram_tensor("bpk", [D], F32, kind="ExternalInput")
    Wpq = nc.dram_tensor("Wpq", [D, D], MM_DT, kind="ExternalInput")
    bpq = nc.dram_tensor("bpq", [D], F32, kind="ExternalInput")
    Wo = nc.dram_tensor("Wo", [D, D], MM_DT, kind="ExternalInput")
    bo = nc.dram_tensor("bo", [D], F32, kind="ExternalInput")
    y = nc.dram_tensor("y", [S, D], F32, kind="ExternalOutput")

    with tile.TileContext(nc) as tc:
        with (
            tc.tile_pool(name="const", bufs=1) as constp,
            tc.tile_pool(name="persist", bufs=1) as persist,
        ):
            ident = constp.tile([128, 128], F32, name="ident")
            make_identity(nc, ident)
            ident_r = constp.tile([128, 128], MM_DT, name="ident_r")
            nc.scalar.copy(ident_r[:], ident[:])

            # =========================== phase A ===========================
            with (
                tc.tile_pool(name="wload", bufs=1) as wload,
                tc.tile_pool(name="ps_xt", bufs=1, space="PSUM") as ps_xt,
                tc.tile_pool(name="ps_rt", bufs=2, space="PSUM") as ps_rt,
                tc.tile_pool(name="ps_pj", bufs=3, space="PSUM") as ps_pj,
            ):

                def load_packed(dram, nrows, name, dt=F32, eng=None):
                    """One big DMA: [nrows, D] row chunks packed side by side
                    in the free dim; chunk c = tile[:, c*D:(c+1)*D]."""
                    eng = eng or nc.sync
                    nch = (nrows + 127) // 128
                    t = wload.tile([128, nch * D], dt, name=name)
                    full = nrows // 128
                    flat = dram[:, :].rearrange("a b -> (a b)")
                    rs = t.ap[0][0]
                    if full:
                        eng.dma_start(
                            AP(t.tensor, t.offset, [[rs, 128], [D, full], [1, D]]),
                            AP(flat.tensor, 0, [[D, 128], [128 * D, full], [1, D]]),
                        )
                    if full < nch:  # remainder rows
                        p = nrows - full * 128
                        eng.dma_start(
                            t[:p, full * D : full * D + D],
                            dram[full * 128 : nrows, :],
                        )
                    return [t[:, c * D : (c + 1) * D] for c in range(nch)]

                def load_chunked(dram, nrows, name, eng, dt=MM_DT):
                    nch = (nrows + 127) // 128
                    t = wload.tile([128, nch * D], dt, name=name)
                    for c in range(nch):
                        p = min(128, nrows - c * 128)
                        eng.dma_start(
                            t[:p, c * D : c * D + D],
                            dram[c * 128 : c * 128 + p, :],
                        )
                    return [t[:, c * D : (c + 1) * D] for c in range(nch)]


                bv_bc = constp.tile([128, D], F32, name="bv_bc")
                nc.sync.dma_start(bv_bc[:], AP(bv[:].tensor, 0, [[0, 128], [1, D]]))

                x_t = load_chunked(x, S, "x", eng=nc.scalar)
                rel_t = load_chunked(rel_tab, W, "rel", eng=nc.scalar)
                Wq_t = load_chunked(Wq, D, "Wq", eng=nc.sync)
                Wk_t = load_chunked(Wk, D, "Wk", eng=nc.sync)
                Wv_t = load_chunked(Wv, D, "Wv", eng=nc.sync)
                Wpk_t = load_chunked(Wpk, D, "Wpk", eng=nc.sync)
                Wpq_t = load_chunked(Wpq, D, "Wpq", eng=nc.sync)

                def load_bias_cols(dram, name):
                    t = constp.tile([128, 4], F32, name=name)
                    rs = t.ap[0][0]
                    nc.sync.dma_start(
                        AP(t.tensor, t.offset, [[rs, 128], [1, 4], [1, 1]]),
                        AP(dram[:].tensor, 0, [[1, 128], [128, 4], [1, 1]]),
                    )
                    return [t[:, c : c + 1] for c in range(4)]

                bq_t = load_bias_cols(bq, "bq")
                bk_t = load_bias_cols(bk, "bk")
                bpk_t = load_bias_cols(bpk, "bpk")
                bpq_t = load_bias_cols(bpq, "bpq")

                # ---- xT via PE transpose ----
                xT_t = []
                for ec in range(4):
                    ps = ps_xt.tile([128, S], F32, name="ps_xtt", tag="ps_xtt")
                    for sc in range(4):
                        nc.tensor.matmul(
                            ps[:, sc * 128 : (sc + 1) * 128].bitcast(MM_DT),
                            x_t[sc][:, ec * 128 : (ec + 1) * 128],
                            ident_r[:],
                            is_transpose=True,
                            start=(sc == 0),
                            stop=(sc == 3),
                        )
                    t = wload.tile([128, S], MM_DT, name=f"xT{ec}")
                    nc.scalar.copy(t[:], ps[:])
                    xT_t.append(t)

                # ---- qT, kT (per-partition bias), v (broadcast bias) ----
                def proj_T(W_t, b_t, name):
                    out = []
                    for dcc in range(4):
                        ps = ps_pj.tile([128, S], F32, name="ps_prj", tag="ps_prj")
                        for ec in range(4):
                            nc.tensor.matmul(
                                ps[:],
                                W_t[ec][:, dcc * 128 : (dcc + 1) * 128],
                                xT_t[ec][:],
                                start=(ec == 0),
                                stop=(ec == 3),
                            )
                        t = persist.tile([128, S], MM_DT, name=f"{name}{dcc}")
                        nc.scalar.activation(
                            t[:],
                            ps[:],
                            mybir.ActivationFunctionType.Identity,
                            bias=b_t[dcc],
                        )
                        out.append(t)
                    return out

                qT_t = proj_T(Wq_t, bq_t, "qT")
                kT_t = proj_T(Wk_t, bk_t, "kT")

                v_t = []
                for sc in range(4):
                    ps = ps_pj.tile([128, D], F32, name="ps_vv", tag="ps_prj")
                    for ec in range(4):
                        nc.tensor.matmul(
                            ps[:],
                            xT_t[ec][:, sc * 128 : (sc + 1) * 128],
                            Wv_t[ec][:],
                            start=(ec == 0),
                            stop=(ec == 3),
                        )
                    t = persist.tile([128, D], F32, name=f"v{sc}")
                    nc.vector.tensor_add(t[:], ps[:], bv_bc[:])
                    v_t.append(t)

                # all heads' ones-augmented v built once, off the head loop
                vh_all = []
                for h in range(H):
                    row = []
                    for sc in range(4):
                        va = persist.tile(
                            [128, DH + 1], MM_DT, name=f"vaug{h}_{sc}"
                        )
                        nc.vector.tensor_copy(
                            va[:, 0:DH], v_t[sc][:, h * DH : h * DH + DH]
                        )
                        nc.vector.memset(va[:, DH : DH + 1].bitcast(F32), 1.0)
                        row.append(va)
                    vh_all.append(row)

                # ---- rel_tabT via PE transpose: [512, 1023] ----
                relT_t = []
                for dc in range(4):
                    ps = ps_rt.tile([128, WP], F32, name="ps_rtt", tag="ps_rtt")
                    for rc in range(8):
                        # last chunk has 127 valid rows; transpose all 128 --
                        # the garbage column lands in the pad col 1023, which
                        # the eviction below never reads.
                        nc.tensor.matmul(
                            ps[:, rc * 128 : rc * 128 + 128].bitcast(MM_DT),
                            rel_t[rc][:, dc * 128 : (dc + 1) * 128],
                            ident_r[:],
                            is_transpose=True,
                            start=(rc % 4 == 0),
                            stop=(rc % 4 == 3),
                        )
                    t = wload.tile([128, WP], MM_DT, name=f"relT{dc}")
                    if dc % 2 == 0:
                        nc.vector.tensor_copy(t[:, 0:W], ps[:, 0:W])
                    else:
                        nc.scalar.copy(t[:, 0:W], ps[:, 0:W])
                    nc.vector.memset(t[:, W:WP].bitcast(F32), 0.0)
                    relT_t.append(t)

                # ---- P_kT [512, 1024] and P_qT flipped ----
                def posproj_chunk(W_t, b_t, name, flip, dcc):
                    ps = ps_rt.tile([128, WP], F32, name="ps_pp", tag="ps_rtt")
                    for n0 in (0, 512):
                        for ec in range(4):
                            nc.tensor.matmul(
                                ps[:, n0 : n0 + 512],
                                W_t[ec][:, dcc * 128 : (dcc + 1) * 128],
                                relT_t[ec][:, n0 : n0 + 512],
                                start=(ec == 0),
                                stop=(ec == 3),
                            )
                    t = persist.tile([128, WP], MM_DT, name=f"{name}{dcc}")
                    if flip:
                        nc.scalar.activation(
                            t[:, 0:W],
                            _rev_ap(ps, W),
                            mybir.ActivationFunctionType.Identity,
                            bias=b_t[dcc],
                        )
                        nc.vector.memset(t[:, W:WP].bitcast(F32), 0.0)
                    else:
                        nc.scalar.activation(
                            t[:],
                            ps[:],
                            mybir.ActivationFunctionType.Identity,
                            bias=b_t[dcc],
                        )
                    return t

                PkT_t, PqTf_t = [], []
                for dcc in range(4):
                    PkT_t.append(
                        posproj_chunk(Wpk_t, bpk_t, "PkT", False, dcc)
                    )
                    PqTf_t.append(
                        posproj_chunk(Wpq_t, bpq_t, "PqTf", True, dcc)
                    )

                bo_bc = constp.tile([128, D], F32, name="bo_bc")
                nc.sync.dma_start(bo_bc[:], AP(bo[:].tensor, 0, [[0, 128], [1, D]]))
                Wo_pk = persist.tile([DH, H * D], MM_DT, name="Wo_pk")
                for h in range(H):
                    nc.sync.dma_start(
                        Wo_pk[:, h * D : (h + 1) * D],
                        Wo[h * DH : (h + 1) * DH, :],
                    )
                Wo_h = [Wo_pk[:, h * D : (h + 1) * D] for h in range(H)]

            # =========================== phase B ===========================
            # Heads in pairs: even head on partitions 0-63, odd on 64-127,
            # so their K=64 matmuls pack into disjoint PE row groups.
            with (
                tc.tile_pool(name="hwork", bufs=2) as hwork,
                tc.tile_pool(name="ps_h", bufs=6, space="PSUM") as ps_h,
                tc.tile_pool(name="ps_av", bufs=2, space="PSUM") as ps_av,
            ):

                def qp_pipeline(thT, PhT, tag, dt):
                    """qP/kPf window halves -> evict -> diag read.
                    Only the 640-wide window [384-i0, 1024-i0) of qP row-chunk
                    ic is ever read by the diagonal, so compute just that:
                    one N=512 and one N=128 matmul per chunk.
                    Returns 4 diag tiles [128, S]."""
                    outs = []
                    for ic in range(4):
                        i0 = ic * 128
                        pa = ps_h.tile([128, 512], F32, name="ps_qpa", tag="ps_h")
                        nc.tensor.matmul(
                            pa[:],
                            thT[:, i0 : i0 + 128],
                            PhT[:, 384 - i0 : 896 - i0],
                        )
                        pb = ps_h.tile([128, 128], F32, name="ps_qpb", tag="ps_h")
                        nc.tensor.matmul(
                            pb[:],
                            thT[:, i0 : i0 + 128],
                            PhT[:, 896 - i0 : 1024 - i0],
                        )
                        sb = hwork.tile([128, WIN], dt, name=f"{tag}sb{ic}",
                                        bufs=2 if tag == "qp" else 1)
                        if ic % 2 == 0:
                            nc.vector.tensor_copy(sb[:, 0:512], pa[:])
                            nc.scalar.copy(sb[:, 512:640], pb[:])
                        else:
                            nc.scalar.copy(sb[:, 0:512], pa[:])
                            nc.vector.tensor_copy(sb[:, 512:640], pb[:])
                        dg = hwork.tile([128, S], dt, name=f"{tag}dg{ic}", bufs=3)
                        nc.sync.dma_start(dg[:], _diag_ap(sb, 127, 128, S))
                        outs.append(dg)
                    return outs

                outT_h = [None] * H

                def head_views(h):
                    dc, hs = h // 2, (h % 2) * DH
                    return (
                        qT_t[dc][hs : hs + DH, :],
                        kT_t[dc][hs : hs + DH, :],
                        PkT_t[dc][hs : hs + DH, :],
                        PqTf_t[dc][hs : hs + DH, :],
                    )

                def emit_pipes(h):
                    qhT, khT, PkhT, PqhTf = head_views(h)
                    c2p = qp_pipeline(qhT, PkhT, "qp", MM_DT)
                    p2cT = qp_pipeline(khT, PqhTf, "kp", BF16)
                    return c2p, p2cT

                pipes = {0: emit_pipes(0)}
                for h in range(H):
                    qhT, khT, PkhT, PqhTf = head_views(h)
                    if h + 1 < H:
                        pipes[h + 1] = emit_pipes(h + 1)
                    c2p, p2cT = pipes.pop(h)

                    ex = []
                    for jc in range(4):
                        ps = ps_h.tile([128, S], F32, name="ps_lg", tag="ps_h")
                        nc.tensor.matmul(
                            ps[:],
                            khT[:, jc * 128 : (jc + 1) * 128],
                            qhT[:],
                            start=True,
                            stop=False,
                        )
                        for ic in range(4):
                            nc.tensor.matmul(
                                ps[:, ic * 128 : (ic + 1) * 128].bitcast(MM_DT),
                                c2p[ic][:, jc * 128 : (jc + 1) * 128],
                                ident_r[:],
                                is_transpose=True,
                                start=False,
                                stop=(ic == 3),
                            )
                        et = hwork.tile([128, S], MM_DT, name=f"ex{jc}", bufs=3)
                        nc.vector.tensor_add(et[:], ps[:], p2cT[jc][:])
                        nc.scalar.activation(
                            et[:],
                            et[:],
                            mybir.ActivationFunctionType.Exp,
                            scale=SCALE,
                        )
                        ex.append(et[:])

                    av = ps_av.tile([DH + 1, S], F32, name="ps_avt", tag="ps_avt")
                    for jc in range(4):
                        nc.tensor.matmul(
                            av[:],
                            vh_all[h][jc][:],
                            ex[jc],
                            start=(jc == 0),
                            stop=(jc == 3),
                        )
                    zr = hwork.tile([DH + 1, S], F32, name="zrow")
                    nc.vector.reciprocal(zr[DH : DH + 1, :], av[DH : DH + 1, :])
                    rbc = hwork.tile([DH, S], F32, name="rbc")
                    rs_zr = zr.ap[0][0]
                    nc.sync.dma_start(
                        rbc[:],
                        AP(zr.tensor, zr.offset + DH * rs_zr,
                           [[rs_zr, 1], [0, DH], [1, S]]),
                    )
                    ot = persist.tile([DH, S], MM_DT, name=f"outT{h}")
                    nc.vector.tensor_mul(ot[:], av[0:DH, :], rbc[:])
                    outT_h[h] = ot

                # ======================= phase C ==========================
                ysb = hwork.tile([128, 4 * D], F32, name="ysb", bufs=1)
                for sc in range(4):
                    ps = ps_h.tile([128, D], F32, name="ps_y", tag="ps_h")
                    for h in range(H):
                        nc.tensor.matmul(
                            ps[:],
                            outT_h[h][:, sc * 128 : (sc + 1) * 128],
                            Wo_h[h],
                            start=(h == 0),
                            stop=(h == 7),
                        )
                    nc.vector.tensor_add(
                        ysb[:, sc * D : (sc + 1) * D], ps[:], bo_bc[:]
                    )
                    nc.sync.dma_start(
                        y[sc * 128 : (sc + 1) * 128, :],
                        ysb[:, sc * D : (sc + 1) * D],
                    )

    nc.compile()
    return nc


_cache_lock = threading.Lock()
_cached_nc = None


def _get_program():
    global _cached_nc
    with _cache_lock:
        if _cached_nc is None:
            _cached_nc = build_program()
    return _cached_nc


def kernel(**inputs):
    x = np.ascontiguousarray(np.asarray(inputs["x"], dtype=np.float32))
    B = x.shape[0]
    assert x.shape == (B, S, D)

    weights = {
        k: np.ascontiguousarray(np.asarray(inputs[k], dtype=np.float32))
        for k in (
            "Wq", "bq", "Wk", "bk", "Wv", "bv",
            "rel_tab", "Wpk", "bpk", "Wpq", "bpq", "Wo", "bo",
        )
    }

    nc = _get_program()
    in_maps = [{"x": x[c], **weights} for c in range(NCORES)]
    res = run_bass_kernel_spmd(nc, in_maps, core_ids=list(range(NCORES)))
    out = np.stack([res.results[c]["y"] for c in range(NCORES)], axis=0)
    return out.astype(np.float32)


if __name__ == "__main__":
    rng = np.random.default_rng(0)
    ins = {
        "x": rng.standard_normal((NCORES, S, D), dtype=np.float32),
        "rel_tab": rng.standard_normal((W, D), dtype=np.float32),
    }
    for nm in ("Wq", "Wk", "Wv", "Wpk", "Wpq", "Wo"):
        ins[nm] = rng.standard_normal((D, D), dtype=np.float32) * 0.04
    for nm in ("bq", "bk", "bv", "bpk", "bpq", "bo"):
        ins[nm] = rng.standard_normal(D).astype(np.float32) * 0.01
    out = kernel(**ins)
    print("ran:", out.shape, out.dtype, np.abs(out).max())



# revision 64
# speedup vs baseline: 1.0021x; 1.0008x over previous
"""Disentangled multi-head attention (DeBERTa-style) Trainium2 Bass kernel.

Full inputs in, full outputs out. Sharding: batch (B=8) across 8 cores, data
parallel; each core computes all H=8 heads for its batch element.

Math (per batch b):
  q,k,v = x@W? + b?                                   [S, D]
  rel_emb[i,j] = rel_tab[j-i+511]  (Toeplitz: only 1023 distinct rows)
  P_k = rel_tab@Wpk + bpk ; P_q = rel_tab@Wpq + bpq   [1023, D]
  c2c[i,j] = q_i . k_j
  c2p[i,j] = q_i . P_k[j-i+511]  = qP[i, j-i+511],    qP  = q @ P_k^T
  p2c[i,j] = k_j . P_q[j-i+511]  = kPf[j, i-j+511],   kPf = k @ P_qflip^T
  out = softmax((c2c+c2p+p2c)/sqrt(3*64)) @ v ; y = out@Wo + bo

Kernel works in transposed-logits layout logitsT[j, i]:
  c2cT  : matmul(lhsT=khT_chunk, rhs=qhT)
  c2pT  : diag-DMA qP rows (per-partition shifted slice) then PE-transpose
  p2cT  : diag-DMA kPf rows directly (already [j, i])
  softmax: exp on ACT; denominator via ones-column in the AV matmul
  (row 64 of av psum = sum_j expT[j,i]); normalize after AV.

Heads are software-pipelined (head h+1's qP/kPf matmul+evict+diag chains
are emitted before head h's logits/AV phase). Adjacent heads sit on PE row
groups 0-63 / 64-127, so their K=64 matmuls can pack into disjoint row
groups at runtime. Dense matmuls stream float32r (1 cyc/row vs 4 for fp32);
qP/kPf compute only the 640-wide window the diagonal actually reads; the
p2cT diag path runs in bf16.
"""

import math
import os
import sys
import threading

import numpy as np

for _p in ("/opt/trn_rl_repo",):
    if _p not in sys.path and os.path.isdir(_p):
        sys.path.insert(0, _p)

import concourse.bacc as bacc
import concourse.bass as bass
import concourse.mybir as mybir
import concourse.tile as tile
from concourse.ap import AP
from concourse.bass_utils import run_bass_kernel_spmd
from concourse.masks import make_identity

S = 512
D = 512
H = 8
DH = 64
L = 512
W = 2 * L - 1  # 1023
WP = 1024  # padded so fp32r matmuls keep even 512-wide moving dims
WIN = 640  # 639-wide diag window, rounded up
NCORES = 8
SCALE = 1.0 / math.sqrt(3.0 * DH)

F32 = mybir.dt.float32
F32R = mybir.dt.float32r
BF16 = mybir.dt.bfloat16
MM_DT = F32R


def _diag_ap(t, col0, nrows, ncols):
    """Per-partition shifted read: out[p, j] = t[p, col0 - p + j]."""
    rs = t.ap[0][0]
    return AP(t.tensor, t.offset + col0, [[rs - 1, nrows], [1, ncols]])


def _rev_ap(t, ncols):
    """Free-dim reversed view of a [P, ncols] tile/psum AP."""
    rs = t.ap[0][0]
    return AP(t.tensor, t.offset + ncols - 1, [[rs, t.shape[0]], [-1, ncols]])


def build_program():
    nc = bacc.Bacc(trn_type="TRN2")

    x = nc.dram_tensor("x", [S, D], MM_DT, kind="ExternalInput")
    Wq = nc.dram_tensor("Wq", [D, D], MM_DT, kind="ExternalInput")
    bq = nc.dram_tensor("bq", [D], F32, kind="ExternalInput")
    Wk = nc.dram_tensor("Wk", [D, D], MM_DT, kind="ExternalInput")
    bk = nc.dram_tensor("bk", [D], F32, kind="ExternalInput")
    Wv = nc.dram_tensor("Wv", [D, D], MM_DT, kind="ExternalInput")
    bv = nc.dram_tensor("bv", [D], F32, kind="ExternalInput")
    rel_tab = nc.dram_tensor("rel_tab", [W, D], MM_DT, kind="ExternalInput")
    Wpk = nc.dram_tensor("Wpk", [D, D], MM_DT, kind="ExternalInput")
    bpk = nc.dram_tensor("bpk", [D], F32, kind="ExternalInput")
    Wpq = nc.dram_tensor("Wpq", [D, D], MM_DT, kind="ExternalInput")
    bpq = nc.dram_tensor("bpq", [D], F32, kind="ExternalInput")
    Wo = nc.dram_tensor("Wo", [D, D], MM_DT, kind="ExternalInput")
    bo = nc.dram_tensor("bo", [D], F32, kind="ExternalInput")
    y = nc.dram_tensor("y", [S, D], F32, kind="ExternalOutput")

    with tile.TileContext(nc) as tc:
        with (
            tc.tile_pool(name="const", bufs=1) as constp,
            tc.tile_pool(name="persist", bufs=1) as persist,
        ):
            ident = constp.tile([128, 128], F32, name="ident")
            make_identity(nc, ident)
            ident_r = constp.tile([128, 128], MM_DT, name="ident_r")
            nc.scalar.copy(ident_r[:], ident[:])

            # =========================== phase A ===========================
            with (
                tc.tile_pool(name="wload", bufs=1) as wload,
                tc.tile_pool(name="ps_xt", bufs=1, space="PSUM") as ps_xt,
                tc.tile_pool(name="ps_rt", bufs=2, space="PSUM") as ps_rt,
                tc.tile_pool(name="ps_pj", bufs=3, space="PSUM") as ps_pj,
            ):

                def load_packed(dram, nrows, name, dt=F32, eng=None):
                    """One big DMA: [nrows, D] row chunks packed side by side
                    in the free dim; chunk c = tile[:, c*D:(c+1)*D]."""
                    eng = eng or nc.sync
                    nch = (nrows + 127) // 128
                    t = wload.tile([128, nch * D], dt, name=name)
                    full = nrows // 128
                    flat = dram[:, :].rearrange("a b -> (a b)")
                    rs = t.ap[0][0]
                    if full:
                        eng.dma_start(
                            AP(t.tensor, t.offset, [[rs, 128], [D, full], [1, D]]),
                            AP(flat.tensor, 0, [[D, 128], [128 * D, full], [1, D]]),
                        )
                    if full < nch:  # remainder rows
                        p = nrows - full * 128
                        eng.dma_start(
                            t[:p, full * D : full * D + D],
                            dram[full * 128 : nrows, :],
                        )
                    return [t[:, c * D : (c + 1) * D] for c in range(nch)]

                def load_chunked(dram, nrows, name, eng, dt=MM_DT):
                    nch = (nrows + 127) // 128
                    t = wload.tile([128, nch * D], dt, name=name)
                    for c in range(nch):
                        p = min(128, nrows - c * 128)
                        eng.dma_start(
                            t[:p, c * D : c * D + D],
                            dram[c * 128 : c * 128 + p, :],
                        )
                    return [t[:, c * D : (c + 1) * D] for c in range(nch)]


                bv_bc = constp.tile([128, D], F32, name="bv_bc")
                nc.sync.dma_start(bv_bc[:], AP(bv[:].tensor, 0, [[0, 128], [1, D]]))

                x_t = load_chunked(x, S, "x", eng=nc.scalar)
                rel_t = load_chunked(rel_tab, W, "rel", eng=nc.scalar)
                Wq_t = load_chunked(Wq, D, "Wq", eng=nc.sync)
                Wk_t = load_chunked(Wk, D, "Wk", eng=nc.sync)
                Wv_t = load_chunked(Wv, D, "Wv", eng=nc.sync)
                Wpk_t = load_chunked(Wpk, D, "Wpk", eng=nc.sync)
                Wpq_t = load_chunked(Wpq, D, "Wpq", eng=nc.sync)

                def load_bias_cols(dram, name):
                    t = constp.tile([128, 4], F32, name=name)
                    rs = t.ap[0][0]
                    nc.sync.dma_start(
                        AP(t.tensor, t.offset, [[rs, 128], [1, 4], [1, 1]]),
                        AP(dram[:].tensor, 0, [[1, 128], [128, 4], [1, 1]]),
                    )
                    return [t[:, c : c + 1] for c in range(4)]

                bq_t = load_bias_cols(bq, "bq")
                bk_t = load_bias_cols(bk, "bk")
                bpk_t = load_bias_cols(bpk, "bpk")
                bpq_t = load_bias_cols(bpq, "bpq")

                # ---- xT via PE transpose ----
                xT_t = []
                for ec in range(4):
                    ps = ps_xt.tile([128, S], F32, name="ps_xtt", tag="ps_xtt")
                    for sc in range(4):
                        nc.tensor.matmul(
                            ps[:, sc * 128 : (sc + 1) * 128].bitcast(MM_DT),
                            x_t[sc][:, ec * 128 : (ec + 1) * 128],
                            ident_r[:],
                            is_transpose=True,
                            start=(sc == 0),
                            stop=(sc == 3),
                        )
                    t = wload.tile([128, S], MM_DT, name=f"xT{ec}")
                    nc.scalar.copy(t[:], ps[:])
                    xT_t.append(t)

                # ---- qT, kT (per-partition bias), v (broadcast bias) ----
                def proj_T(W_t, b_t, name):
                    out = []
                    for dcc in range(4):
                        ps = ps_pj.tile([128, S], F32, name="ps_prj", tag="ps_prj")
                        for ec in range(4):
                            nc.tensor.matmul(
                                ps[:],
                                W_t[ec][:, dcc * 128 : (dcc + 1) * 128],
                                xT_t[ec][:],
                                start=(ec == 0),
                                stop=(ec == 3),
                            )
                        t = persist.tile([128, S], MM_DT, name=f"{name}{dcc}")
                        nc.scalar.activation(
                            t[:],
                            ps[:],
                            mybir.ActivationFunctionType.Identity,
                            bias=b_t[dcc],
                        )
                        out.append(t)
                    return out

                qT_t = proj_T(Wq_t, bq_t, "qT")
                kT_t = proj_T(Wk_t, bk_t, "kT")

                v_t = []
                for sc in range(4):
                    ps = ps_pj.tile([128, D], F32, name="ps_vv", tag="ps_prj")
                    for ec in range(4):
                        nc.tensor.matmul(
                            ps[:],
                            xT_t[ec][:, sc * 128 : (sc + 1) * 128],
                            Wv_t[ec][:],
                            start=(ec == 0),
                            stop=(ec == 3),
                        )
                    t = persist.tile([128, D], F32, name=f"v{sc}")
                    nc.vector.tensor_add(t[:], ps[:], bv_bc[:])
                    v_t.append(t)

                # all heads' ones-augmented v built once, off the head loop
                vh_all = []
                for h in range(H):
                    row = []
                    for sc in range(4):
                        va = persist.tile(
                            [128, DH + 1], MM_DT, name=f"vaug{h}_{sc}"
                        )
                        nc.vector.tensor_copy(
                            va[:, 0:DH], v_t[sc][:, h * DH : h * DH + DH]
                        )
                        nc.vector.memset(va[:, DH : DH + 1].bitcast(F32), 1.0)
                        row.append(va)
                    vh_all.append(row)

                # ---- rel_tabT via PE transpose: [512, 1023] ----
                relT_t = []
                for dc in range(4):
                    ps = ps_rt.tile([128, WP], F32, name="ps_rtt", tag="ps_rtt")
                    for rc in range(8):
                        # last chunk has 127 valid rows; transpose all 128 --
                        # the garbage column lands in the pad col 1023, which
                        # the eviction below never reads.
                        nc.tensor.matmul(
                            ps[:, rc * 128 : rc * 128 + 128].bitcast(MM_DT),
                            rel_t[rc][:, dc * 128 : (dc + 1) * 128],
                            ident_r[:],
                            is_transpose=True,
                            start=(rc % 4 == 0),
                            stop=(rc % 4 == 3),
                        )
                    t = wload.tile([128, WP], MM_DT, name=f"relT{dc}")
                    if dc % 2 == 0:
                        nc.vector.tensor_copy(t[:, 0:W], ps[:, 0:W])
                    else:
                        nc.scalar.copy(t[:, 0:W], ps[:, 0:W])
                    nc.vector.memset(t[:, W:WP].bitcast(F32), 0.0)
                    relT_t.append(t)

                # ---- P_kT [512, 1024] and P_qT flipped ----
                def posproj_chunk(W_t, b_t, name, flip, dcc):
                    ps = ps_rt.tile([128, WP], F32, name="ps_pp", tag="ps_rtt")
                    for n0 in (0, 512):
                        for ec in range(4):
                            nc.tensor.matmul(
                                ps[:, n0 : n0 + 512],
                                W_t[ec][:, dcc * 128 : (dcc + 1) * 128],
                                relT_t[ec][:, n0 : n0 + 512],
                                start=(ec == 0),
                                stop=(ec == 3),
                            )
                    t = persist.tile([128, WP], MM_DT, name=f"{name}{dcc}")
                    if flip:
                        nc.scalar.activation(
                            t[:, 0:W],
                            _rev_ap(ps, W),
                            mybir.ActivationFunctionType.Identity,
                            bias=b_t[dcc],
                        )
                        nc.vector.memset(t[:, W:WP].bitcast(F32), 0.0)
                    else:
                        nc.scalar.activation(
                            t[:],
                            ps[:],
                            mybir.ActivationFunctionType.Identity,
                            bias=b_t[dcc],
                        )
                    return t

                PkT_t, PqTf_t = [], []
                for dcc in range(4):
                    PkT_t.append(
                        posproj_chunk(Wpk_t, bpk_t, "PkT", False, dcc)
                    )
                    PqTf_t.append(
                        posproj_chunk(Wpq_t, bpq_t, "PqTf", True, dcc)
                    )

                bo_bc = constp.tile([128, D], F32, name="bo_bc")
                nc.sync.dma_start(bo_bc[:], AP(bo[:].tensor, 0, [[0, 128], [1, D]]))
                Wo_pk = persist.tile([DH, H * D], MM_DT, name="Wo_pk")
                for h in range(H):
                    nc.sync.dma_start(
                        Wo_pk[:, h * D : (h + 1) * D],
                        Wo[h * DH : (h + 1) * DH, :],
                    )
                Wo_h = [Wo_pk[:, h * D : (h + 1) * D] for h in range(H)]

            # =========================== phase B ===========================
            # Heads in pairs: even head on partitions 0-63, odd on 64-127,
            # so their K=64 matmuls pack into disjoint PE row groups.
            with (
                tc.tile_pool(name="hwork", bufs=2) as hwork,
                tc.tile_pool(name="ps_h", bufs=6, space="PSUM") as ps_h,
                tc.tile_pool(name="ps_av", bufs=2, space="PSUM") as ps_av,
            ):

                def qp_pipeline(thT, PhT, tag, dt):
                    """qP/kPf window halves -> evict -> diag read.
                    Only the 640-wide window [384-i0, 1024-i0) of qP row-chunk
                    ic is ever read by the diagonal, so compute just that:
                    one N=512 and one N=128 matmul per chunk.
                    Returns 4 diag tiles [128, S]."""
                    outs = []
                    for ic in range(4):
                        i0 = ic * 128
                        pa = ps_h.tile([128, 512], F32, name="ps_qpa", tag="ps_h")
                        nc.tensor.matmul(
                            pa[:],
                            thT[:, i0 : i0 + 128],
                            PhT[:, 384 - i0 : 896 - i0],
                        )
                        pb = ps_h.tile([128, 128], F32, name="ps_qpb", tag="ps_h")
                        nc.tensor.matmul(
                            pb[:],
                            thT[:, i0 : i0 + 128],
                            PhT[:, 896 - i0 : 1024 - i0],
                        )
                        sb = hwork.tile([128, WIN], dt, name=f"{tag}sb{ic}",
                                        bufs=2)
                        if ic % 2 == 0:
                            nc.vector.tensor_copy(sb[:, 0:512], pa[:])
                            nc.scalar.copy(sb[:, 512:640], pb[:])
                        else:
                            nc.scalar.copy(sb[:, 0:512], pa[:])
                            nc.vector.tensor_copy(sb[:, 512:640], pb[:])
                        dg = hwork.tile([128, S], dt, name=f"{tag}dg{ic}", bufs=3)
                        nc.sync.dma_start(dg[:], _diag_ap(sb, 127, 128, S))
                        outs.append(dg)
                    return outs

                outT_h = [None] * H

                def head_views(h):
                    dc, hs = h // 2, (h % 2) * DH
                    return (
                        qT_t[dc][hs : hs + DH, :],
                        kT_t[dc][hs : hs + DH, :],
                        PkT_t[dc][hs : hs + DH, :],
                        PqTf_t[dc][hs : hs + DH, :],
                    )

                def emit_pipes(h):
                    qhT, khT, PkhT, PqhTf = head_views(h)
                    c2p = qp_pipeline(qhT, PkhT, "qp", MM_DT)
                    p2cT = qp_pipeline(khT, PqhTf, "kp", BF16)
                    return c2p, p2cT

                pipes = {0: emit_pipes(0)}
                for h in range(H):
                    qhT, khT, PkhT, PqhTf = head_views(h)
                    if h + 1 < H:
                        pipes[h + 1] = emit_pipes(h + 1)
                    c2p, p2cT = pipes.pop(h)

                    ex = []
                    for jc in range(4):
                        ps = ps_h.tile([128, S], F32, name="ps_lg", tag="ps_h")
                        nc.tensor.matmul(
                            ps[:],
                            khT[:, jc * 128 : (jc + 1) * 128],
                            qhT[:],
                            start=True,
                            stop=False,
                        )
                        for ic in range(4):
                            nc.tensor.matmul(
                                ps[:, ic * 128 : (ic + 1) * 128].bitcast(MM_DT),
                                c2p[ic][:, jc * 128 : (jc + 1) * 128],
                                ident_r[:],
                                is_transpose=True,
                                start=False,
                                stop=(ic == 3),
                            )
                        et = hwork.tile([128, S], MM_DT, name=f"ex{jc}", bufs=3)
                        nc.vector.tensor_add(et[:], ps[:], p2cT[jc][:])
                        nc.scalar.activation(
                            et[:],
                            et[:],
                            mybir.ActivationFunctionType.Exp,
                            scale=SCALE,
                        )
                        ex.append(et[:])

                    av = ps_av.tile([DH + 1, S], F32, name="ps_avt", tag="ps_avt")
                    for jc in range(4):
                        nc.tensor.matmul(
                            av[:],
                            vh_all[h][jc][:],
                            ex[jc],
                            start=(jc == 0),
                            stop=(jc == 3),
                        )
                    zr = hwork.tile([DH + 1, S], F32, name="zrow", bufs=1)
                    nc.vector.reciprocal(zr[DH : DH + 1, :], av[DH : DH + 1, :])
                    rbc = hwork.tile([DH, S], F32, name="rbc", bufs=1)
                    rs_zr = zr.ap[0][0]
                    nc.sync.dma_start(
                        rbc[:],
                        AP(zr.tensor, zr.offset + DH * rs_zr,
                           [[rs_zr, 1], [0, DH], [1, S]]),
                    )
                    ot = persist.tile([DH, S], MM_DT, name=f"outT{h}")
                    nc.vector.tensor_mul(ot[:], av[0:DH, :], rbc[:])
                    outT_h[h] = ot

                # ======================= phase C ==========================
                ysb = hwork.tile([128, 4 * D], F32, name="ysb", bufs=1)
                for sc in range(4):
                    ps = ps_h.tile([128, D], F32, name="ps_y", tag="ps_h")
                    for h in range(H):
                        nc.tensor.matmul(
                            ps[:],
                            outT_h[h][:, sc * 128 : (sc + 1) * 128],
                            Wo_h[h],
                            start=(h == 0),
                            stop=(h == 7),
                        )
                    nc.vector.tensor_add(
                        ysb[:, sc * D : (sc + 1) * D], ps[:], bo_bc[:]
                    )
                    nc.sync.dma_start(
                        y[sc * 128 : (sc + 1) * 128, :],
                        ysb[:, sc * D : (sc + 1) * D],
                    )

    nc.compile()
    return nc


_cache_lock = threading.Lock()
_cached_nc = None


def _get_program():
    global _cached_nc
    with _cache_lock:
        if _cached_nc is None:
            _cached_nc = build_program()
    return _cached_nc


def kernel(**inputs):
    x = np.ascontiguousarray(np.asarray(inputs["x"], dtype=np.float32))
    B = x.shape[0]
    assert x.shape == (B, S, D)

    weights = {
        k: np.ascontiguousarray(np.asarray(inputs[k], dtype=np.float32))
        for k in (
            "Wq", "bq", "Wk", "bk", "Wv", "bv",
            "rel_tab", "Wpk", "bpk", "Wpq", "bpq", "Wo", "bo",
        )
    }

    nc = _get_program()
    in_maps = [{"x": x[c], **weights} for c in range(NCORES)]
    res = run_bass_kernel_spmd(nc, in_maps, core_ids=list(range(NCORES)))
    out = np.stack([res.results[c]["y"] for c in range(NCORES)], axis=0)
    return out.astype(np.float32)


if __name__ == "__main__":
    rng = np.random.default_rng(0)
    ins = {
        "x": rng.standard_normal((NCORES, S, D), dtype=np.float32),
        "rel_tab": rng.standard_normal((W, D), dtype=np.float32),
    }
    for nm in ("Wq", "Wk", "Wv", "Wpk", "Wpq", "Wo"):
        ins[nm] = rng.standard_normal((D, D), dtype=np.float32) * 0.04
    for nm in ("bq", "bk", "bv", "bpk", "bpq", "bo"):
        ins[nm] = rng.standard_normal(D).astype(np.float32) * 0.01
    out = kernel(**ins)
    print("ran:", out.shape, out.dtype, np.abs(out).max())

